# revision 16
# baseline (speedup 1.0000x reference)
"""Trainium2 Bass kernel for BitLinear: y[b,s,o] = sum_d x[b,s,d] * w[o,d].

x: [4, 2048, 4096] f32, weight: [4096, 4096] int32 (values 0..255), y f32.

Strategy (v3, mixed fp8/bf16 + 2D sharding):
- 2D shard: 4 token-shards x 2 output-shards -> 2048 tokens x 2048 outs
  per core. Each stationary weight tile serves 4 moving 512-token chunks
  (vs 2 with 1D token sharding), amortizing PE weight-load overhead.
- Contraction dim 4096 split: first N8*256 dims run as fp8e4 DoubleRow
  matmuls (2 contraction planes per MM -> 2x MACs/cycle); the rest runs
  as plain bf16 MMs. Weights 0..255 are exact in bf16; for the fp8 part
  the weight is centered (v = w - 128, |v| <= 128 fits e4m3) and the
  rank-1 term 128 * rowsum(x_fp8dims) is added during PSUM evacuation
  (host computes the rowsum in f64 -> f32, broadcast over partitions).
  x is e4m3 on the fp8 dims, bf16 on the rest. N8 keeps the fp8
  quantization noise well under the 2e-2 gate (measured 1.81e-2 @ N8=12
  on HW, bit-identical to the host sim).
- SWI=1 uses DoubleRowSwInterleave: host pre-interleaves/reverses the
  fp8 weight pairs so the HW weight load is contiguous (FWL-eligible).
- Output groups of 128 features: 4 PSUM banks per group, double
  buffered (8 total) so evacuation overlaps the next group's matmuls.
- Host gathers per-core yt [2048, 2048] f32 -> y[ts tokens, os outs].
"""

import os
import sys

for _p in ("/opt/trn_rl_repo", "/root/.axon_site/_ro/trn_rl_repo"):
    if _p not in sys.path:
        sys.path.append(_p)

import numpy as np
import ml_dtypes

N_CORES = 8
TOKENS = 8192  # 4 * 2048
D_IN = 4096
D_OUT = 4096
TS = 4          # token shards
OS = 2          # output shards
T_SHARD = TOKENS // TS   # 2048
O_SHARD = D_OUT // OS    # 2048
P = 128
MC = T_SHARD // 512      # 4 moving chunks of 512 tokens

# number of 256-wide fp8 DoubleRow k-tiles (0..16); rest of K is bf16
N8 = int(os.environ.get("BL_N8", "12"))
SWI = int(os.environ.get("BL_SWI", "0"))
XI = int(os.environ.get("BL_XI", "0"))  # interleave moving plane pairs
KF8 = N8 * 256
KB = D_IN - KF8
KB_T = KB // P

_NC_CACHE = {}


def _dedupe_ldweights(m):
    """Remove InstLdweights whose weights AP matches the immediately
    preceding load (per block): the Tile legalization pass emits one
    LDWEIGHTS per matmul even when consecutive matmuls share the same
    stationary tile, and the redundant 128/256-column loads serialize on
    the PE weight-load path. Any waits on a removed load are merged into
    the next matmul; loads carrying semaphore updates are kept."""
    removed = 0
    for fn in m.functions:
        for blk in fn.blocks:
            insts = list(blk.instructions)
            keep, last_sig, pending_waits = [], None, []
            for inst in insts:
                n = type(inst).__name__
                if n == "InstLdweights":
                    sig = (str(inst.ins[0]),
                           str(getattr(inst, "perf_mode", None)),
                           str(getattr(inst, "tile_position", None)),
                           str(getattr(inst, "tile_size", None)),
                           str(getattr(inst, "is_transpose", None)))
                    si = inst.sync_info
                    has_upd = bool(si and si.on_update)
                    if sig == last_sig and not has_upd:
                        if si and si.on_wait:
                            pending_waits.extend(list(si.on_wait))
                        removed += 1
                        continue
                    last_sig = sig
                elif n == "InstMatmult":
                    if pending_waits:
                        import concourse.mybir as mybir
                        si = inst.sync_info
                        if si is None:
                            inst.sync_info = mybir.SyncInfo(
                                on_wait=pending_waits, on_update=[])
                        else:
                            si.on_wait = list(si.on_wait) + pending_waits
                        pending_waits = []
                keep.append(inst)
            assert not pending_waits, "dangling waits after ldweights dedupe"
            if len(keep) != len(insts):
                blk.instructions = keep
    return removed


def build_nc(repeats: int = 1):
    """Build (and cache) the Bass program.

    repeats > 1 re-emits the compute body (used only for slope-based HW
    timing; identical output)."""
    key = (N8, SWI, XI, repeats)
    if key in _NC_CACHE:
        return _NC_CACHE[key]

    import concourse.mybir as mybir
    import concourse.tile as tile
    from concourse import bacc

    f8 = mybir.dt.float8e4
    bf16 = mybir.dt.bfloat16
    f32 = mybir.dt.float32
    dr_mode = (mybir.MatmulPerfMode.DoubleRowSwInterleave if SWI
               else mybir.MatmulPerfMode.DoubleRow)

    nc = bacc.Bacc(None, target_bir_lowering=False)
    with tile.TileContext(nc) as tc:
        with tc.tile_pool(name="dram", bufs=1, space="DRAM") as dram:
            if N8:
                # moving layout: XI=0 -> [row, plane, token]; XI=1 ->
                # [row, token, plane] (pair elements adjacent)
                xm8 = dram.tile([N8 * P, T_SHARD, 2] if XI
                                else [N8 * P, 2, T_SHARD], f8,
                                kind="ExternalInput", name="xm8",
                                uniquify=False)
                # weight layout: SWI=0 -> [row, plane, out]; SWI=1 ->
                # [row, out-block, 256 interleaved pair elems]
                if SWI:
                    xn8 = dram.tile([N8 * P, O_SHARD // P, 2 * P], f8,
                                    kind="ExternalInput", name="xn8",
                                    uniquify=False)
                else:
                    xn8 = dram.tile([N8 * P, 2, O_SHARD], f8,
                                    kind="ExternalInput", name="xn8",
                                    uniquify=False)
                basem = dram.tile([P, T_SHARD], f32,
                                  kind="ExternalInput", name="basem",
                                  uniquify=False)
            if KB:
                xm16 = dram.tile([KB, T_SHARD], bf16,
                                 kind="ExternalInput", name="xm16",
                                 uniquify=False)
                xn16 = dram.tile([KB, O_SHARD], bf16,
                                 kind="ExternalInput", name="xn16",
                                 uniquify=False)
                xm16v = xm16[:].rearrange("(kb p) m -> p kb m", p=P)
            yt = dram.tile([O_SHARD, T_SHARD], f32,
                           kind="ExternalOutput", name="yt", uniquify=False)

            with tc.tile_pool(name="xpool", bufs=N8 + KB_T + 1) as xpool, \
                 tc.tile_pool(name="wpool", bufs=8) as wpool, \
                 tc.tile_pool(name="pspool", bufs=2, space="PSUM") as pspool, \
                 tc.tile_pool(name="evpool", bufs=8) as evpool:
                xt8 = [None] * N8
                xt16 = [None] * KB_T
                base_sb = None
                NG = O_SHARD // P   # 16 groups of 128 output features
                first = True
                for _ in range(repeats):
                    for ng in range(NG):
                        banks = [pspool.tile([P, 512], f32,
                                             name=f"bank_{mc}",
                                             tag=f"bank_{mc}")
                                 for mc in range(MC)]
                        if first and N8:
                            base_sb = xpool.tile([P, T_SHARD], f32,
                                                 name="base_sb", tag="xt")
                            nc.sync.dma_start(base_sb[:], basem[:])
                        for kt in range(N8):
                            if SWI:
                                wt8 = wpool.tile([P, 2 * P], f8,
                                                 name="wt8", tag="wt")
                                src = xn8[kt * P:(kt + 1) * P, ng, :]
                            else:
                                wt8 = wpool.tile([P, 2, P], f8,
                                                 name="wt8", tag="wt")
                                src = xn8[kt * P:(kt + 1) * P, :,
                                          ng * P:(ng + 1) * P]
                            nc.sync.dma_start(wt8[:], src)
                            if first:
                                xt = xpool.tile(
                                    [P, T_SHARD, 2] if XI else [P, 2, T_SHARD],
                                    f8, name="xt8", tag="xt")
                                nc.sync.dma_start(
                                    xt[:], xm8[kt * P:(kt + 1) * P, :, :])
                                if XI:
                                    xt8[kt] = xt[:].rearrange(
                                        "p m two -> p two m")
                                else:
                                    xt8[kt] = xt
                            for mc in range(MC):
                                nc.tensor.matmul(
                                    banks[mc][:],
                                    wt8[:],
                                    xt8[kt][:, :, mc * 512:(mc + 1) * 512],
                                    start=(kt == 0),
                                    stop=(KB == 0 and kt == N8 - 1),
                                    perf_mode=dr_mode,
                                )
                        for kb in range(KB_T):
                            wt = wpool.tile([P, P], bf16,
                                            name="wt", tag="wt")
                            nc.sync.dma_start(
                                wt[:], xn16[kb * P:(kb + 1) * P,
                                            ng * P:(ng + 1) * P])
                            if first:
                                xt = xpool.tile([P, T_SHARD], bf16,
                                                name="xt16", tag="xt")
                                nc.sync.dma_start(xt[:], xm16v[:, kb])
                                xt16[kb] = xt
                            for mc in range(MC):
                                nc.tensor.matmul(
                                    banks[mc][:],
                                    wt[:],
                                    xt16[kb][:, mc * 512:(mc + 1) * 512],
                                    start=(N8 == 0 and kb == 0),
                                    stop=(kb == KB_T - 1),
                                )
                        first = False
                        for mc in range(MC):
                            ev = evpool.tile([P, 512], f32,
                                             name="ev", tag="ev")
                            if N8:
                                nc.vector.scalar_tensor_tensor(
                                    out=ev[:],
                                    in0=banks[mc][:],
                                    scalar=0.0,
                                    in1=base_sb[:, mc * 512:(mc + 1) * 512],
                                    op0=mybir.AluOpType.bypass,
                                    op1=mybir.AluOpType.add,
                                )
                            else:
                                nc.vector.tensor_copy(
                                    out=ev[:], in_=banks[mc][:])
                            nc.sync.dma_start(
                                yt[ng * P:(ng + 1) * P,
                                   mc * 512:(mc + 1) * 512],
                                ev[:])
    _dedupe_ldweights(nc.m)
    nc.compile()
    _NC_CACHE[key] = nc
    return nc


def prepare_in_maps(x: np.ndarray, weight: np.ndarray):
    """Host-side shard prep; see module docstring for the layout."""
    bf16 = ml_dtypes.bfloat16
    f8 = ml_dtypes.float8_e4m3
    x2 = np.ascontiguousarray(np.asarray(x).reshape(TOKENS, D_IN))
    w = np.asarray(weight)

    xm8_full = xn8_os = base = None
    xm16_full = xn16_os = None
    if N8:
        xs8 = x2[:, :KF8].astype(f8)  # [TOKENS, KF8]
        v8 = (w[:, :KF8].astype(np.float32) - 128.0).astype(f8)  # [D_OUT,KF8]
        base = (128.0 * x2[:, :KF8].astype(np.float64).sum(axis=1)
                ).astype(np.float32)  # [TOKENS]
        # row r = kt*128+p maps to contraction dim kt*256 + i*128 + p
        if XI:
            xm8_full = np.ascontiguousarray(
                xs8.T.reshape(N8, 2, P, TOKENS).transpose(0, 2, 3, 1)
                .reshape(N8 * P, TOKENS, 2))
        else:
            xm8_full = np.ascontiguousarray(
                xs8.T.reshape(N8, 2, P, TOKENS).transpose(0, 2, 1, 3)
                .reshape(N8 * P, 2, TOKENS))
        w8 = (v8.T.reshape(N8, 2, P, D_OUT).transpose(0, 2, 1, 3)
              .reshape(N8 * P, 2, D_OUT))  # [row, plane, out]
        if SWI:
            # per 128-out block: pairs (plane0, plane1) interleaved with
            # the block's output columns reversed:
            #   elem[2j+i] = plane_i[127 - j]
            w8b = w8.reshape(N8 * P, 2, D_OUT // P, P)  # [row, i, blk, o]
            w8b = w8b[:, :, :, ::-1]                    # reverse cols
            w8b = w8b.transpose(0, 2, 3, 1)             # [row, blk, j, i]
            w8swi = w8b.reshape(N8 * P, D_OUT // P, 2 * P)
            nblk = O_SHARD // P
            xn8_os = [
                np.ascontiguousarray(w8swi[:, o * nblk:(o + 1) * nblk, :])
                for o in range(OS)
            ]
        else:
            xn8_os = [
                np.ascontiguousarray(w8[:, :, o * O_SHARD:(o + 1) * O_SHARD])
                for o in range(OS)
            ]
    if KB:
        xm16_full = np.ascontiguousarray(
            x2[:, KF8:].astype(bf16).T)  # [KB, TOKENS]
        wt16 = w[:, KF8:].astype(np.float32).astype(bf16).T  # [KB, D_OUT]
        xn16_os = [
            np.ascontiguousarray(wt16[:, o * O_SHARD:(o + 1) * O_SHARD])
            for o in range(OS)
        ]

    in_maps = []
    for c in range(N_CORES):
        ts, osd = c % TS, c // TS
        sl = slice(ts * T_SHARD, (ts + 1) * T_SHARD)
        m = {}
        if N8:
            m["xm8"] = np.ascontiguousarray(
                xm8_full[:, sl, :] if XI else xm8_full[:, :, sl])
            m["xn8"] = xn8_os[osd]
            m["basem"] = np.ascontiguousarray(
                np.broadcast_to(base[sl], (P, T_SHARD)))
        if KB:
            m["xm16"] = np.ascontiguousarray(xm16_full[:, sl])
            m["xn16"] = xn16_os[osd]
        in_maps.append(m)
    return in_maps


def gather_output(results):
    y = np.empty((TOKENS, D_OUT), dtype=np.float32)
    for c in range(N_CORES):
        ts, osd = c % TS, c // TS
        y[ts * T_SHARD:(ts + 1) * T_SHARD,
          osd * O_SHARD:(osd + 1) * O_SHARD] = results[c]["yt"].T
    return y.reshape(4, 2048, D_OUT)


def kernel(x: np.ndarray, weight: np.ndarray) -> np.ndarray:
    from concourse.bass_utils import run_bass_kernel_spmd

    nc = build_nc()
    in_maps = prepare_in_maps(x, weight)
    res = run_bass_kernel_spmd(nc, in_maps, core_ids=list(range(N_CORES)))
    return gather_output(res.results)


# revision 20
# speedup vs baseline: 1.0711x; 1.0711x over previous
"""Trainium2 Bass kernel for BitLinear: y[b,s,o] = sum_d x[b,s,d] * w[o,d].

x: [4, 2048, 4096] f32, weight: [4096, 4096] int32 (values 0..255), y f32.

Strategy (v3, mixed fp8/bf16 + 2D sharding):
- 2D shard: 4 token-shards x 2 output-shards -> 2048 tokens x 2048 outs
  per core. Each stationary weight tile serves 4 moving 512-token chunks
  (vs 2 with 1D token sharding), amortizing PE weight-load overhead.
- Contraction dim 4096 split: first N8*256 dims run as fp8e4 DoubleRow
  matmuls (2 contraction planes per MM -> 2x MACs/cycle); the rest runs
  as plain bf16 MMs. Weights 0..255 are exact in bf16; for the fp8 part
  the weight is centered (v = w - 128, |v| <= 128 fits e4m3) and the
  rank-1 term 128 * rowsum(x_fp8dims) is added during PSUM evacuation
  (host computes the rowsum in f64 -> f32, broadcast over partitions).
  x is e4m3 on the fp8 dims, bf16 on the rest. N8 keeps the fp8
  quantization noise well under the 2e-2 gate (measured 1.81e-2 @ N8=12
  on HW, bit-identical to the host sim).
- SWI=1 uses DoubleRowSwInterleave: host pre-interleaves/reverses the
  fp8 weight pairs so the HW weight load is contiguous (FWL-eligible).
- Output groups of 128 features: 4 PSUM banks per group, double
  buffered (8 total) so evacuation overlaps the next group's matmuls.
- Host gathers per-core yt [2048, 2048] f32 -> y[ts tokens, os outs].
"""

import os
import sys

for _p in ("/opt/trn_rl_repo", "/root/.axon_site/_ro/trn_rl_repo"):
    if _p not in sys.path:
        sys.path.append(_p)

import numpy as np
import ml_dtypes

N_CORES = 8
TOKENS = 8192  # 4 * 2048
D_IN = 4096
D_OUT = 4096
TS = 4          # token shards
OS = 2          # output shards
T_SHARD = TOKENS // TS   # 2048
O_SHARD = D_OUT // OS    # 2048
P = 128
MC = T_SHARD // 512      # 4 moving chunks of 512 tokens

# number of 256-wide fp8 DoubleRow k-tiles (0..16); rest of K is bf16
N8 = int(os.environ.get("BL_N8", "12"))
SWI = int(os.environ.get("BL_SWI", "0"))
XI = int(os.environ.get("BL_XI", "0"))  # interleave moving plane pairs
STRIP = int(os.environ.get("BL_STRIP", "0"))  # strip per-MM sem incs
KF8 = N8 * 256
KB = D_IN - KF8
KB_T = KB // P

_NC_CACHE = {}


def _dedupe_ldweights(m):
    """Remove InstLdweights whose weights AP matches the immediately
    preceding load (per block): the Tile legalization pass emits one
    LDWEIGHTS per matmul even when consecutive matmuls share the same
    stationary tile, and the redundant 128/256-column loads serialize on
    the PE weight-load path. Any waits on a removed load are merged into
    the next matmul; loads carrying semaphore updates are kept."""
    removed = 0
    for fn in m.functions:
        for blk in fn.blocks:
            insts = list(blk.instructions)
            keep, last_sig, pending_waits = [], None, []
            for inst in insts:
                n = type(inst).__name__
                if n == "InstLdweights":
                    sig = (str(inst.ins[0]),
                           str(getattr(inst, "perf_mode", None)),
                           str(getattr(inst, "tile_position", None)),
                           str(getattr(inst, "tile_size", None)),
                           str(getattr(inst, "is_transpose", None)))
                    si = inst.sync_info
                    has_upd = bool(si and si.on_update)
                    if sig == last_sig and not has_upd:
                        if si and si.on_wait:
                            pending_waits.extend(list(si.on_wait))
                        removed += 1
                        continue
                    last_sig = sig
                elif n == "InstMatmult":
                    if pending_waits:
                        import concourse.mybir as mybir
                        si = inst.sync_info
                        if si is None:
                            inst.sync_info = mybir.SyncInfo(
                                on_wait=pending_waits, on_update=[])
                        else:
                            si.on_wait = list(si.on_wait) + pending_waits
                        pending_waits = []
                keep.append(inst)
            assert not pending_waits, "dangling waits after ldweights dedupe"
            if len(keep) != len(insts):
                blk.instructions = keep
    return removed


def _strip_mm_sem_incs(m):
    """Keep the PE-semaphore increment only on stop_tensor_calc matmuls
    and renumber every wait on that semaphore to the next kept
    increment. Waiters then sync on PSUM accumulation-group boundaries
    instead of individual matmuls, removing ~95% of the per-MM semaphore
    updates. Requires the weight pool to be deep enough that buffer
    reuse spans a full accumulation group (no intra-group reuse waits),
    else this deadlocks."""
    # locate the single PE semaphore incremented by matmuls and the
    # global matmul order (all matmuls live in one block)
    mms = []
    sem_ids = set()
    for fn in m.functions:
        for blk in fn.blocks:
            for inst in blk.instructions:
                if type(inst).__name__ == "InstMatmult":
                    mms.append(inst)
                    si = inst.sync_info
                    if si:
                        for u in si.on_update:
                            if u.update_mode == "sem-inc":
                                sem_ids.add(u.id)
    if not mms or len(sem_ids) != 1:
        return 0
    sem_id = sem_ids.pop()
    n = len(mms)
    kept = [bool(inst.stop_tensor_calc) for inst in mms]
    assert kept[-1], "last matmul must be a stop MM"
    # old wait value v (1-based count) -> new value: count of kept incs
    # up to the first kept MM at old index >= v
    kept_cum = []
    c = 0
    for k in kept:
        c += k
        kept_cum.append(c)
    next_kept_cum = [0] * (n + 2)
    nxt = None
    for i in range(n - 1, -1, -1):
        if kept[i]:
            nxt = kept_cum[i]
        next_kept_cum[i + 1] = nxt
    removed = 0
    for inst in mms:
        si = inst.sync_info
        if si and not inst.stop_tensor_calc:
            upd = [u for u in si.on_update
                   if not (u.id == sem_id and u.update_mode == "sem-inc")]
            if len(upd) != len(si.on_update):
                si.on_update = upd
                removed += 1
    for fn in m.functions:
        for blk in fn.blocks:
            for inst in blk.instructions:
                si = inst.sync_info
                if not si:
                    continue
                for w in si.on_wait:
                    if w.id == sem_id and w.wait_mode == "sem-ge-imm":
                        v = w.wait_value
                        assert 1 <= v <= n, (v, n)
                        w.wait_value = next_kept_cum[v]
    return removed


def build_nc(repeats: int = 1):
    """Build (and cache) the Bass program.

    repeats > 1 re-emits the compute body (used only for slope-based HW
    timing; identical output)."""
    key = (N8, SWI, XI, STRIP, repeats)
    if key in _NC_CACHE:
        return _NC_CACHE[key]

    import concourse.mybir as mybir
    import concourse.tile as tile
    from concourse import bacc

    f8 = mybir.dt.float8e4
    bf16 = mybir.dt.bfloat16
    f32 = mybir.dt.float32
    dr_mode = (mybir.MatmulPerfMode.DoubleRowSwInterleave if SWI
               else mybir.MatmulPerfMode.DoubleRow)

    nc = bacc.Bacc(None, target_bir_lowering=False)
    with tile.TileContext(nc) as tc:
        with tc.tile_pool(name="dram", bufs=1, space="DRAM") as dram:
            if N8:
                # moving layout: XI=0 -> [row, plane, token]; XI=1 ->
                # [row, token, plane] (pair elements adjacent)
                xm8 = dram.tile([N8 * P, T_SHARD, 2] if XI
                                else [N8 * P, 2, T_SHARD], f8,
                                kind="ExternalInput", name="xm8",
                                uniquify=False)
                # weight layout: SWI=0 -> [row, plane, out]; SWI=1 ->
                # [row, out-block, 256 interleaved pair elems]
                if SWI:
                    xn8 = dram.tile([N8 * P, O_SHARD // P, 2 * P], f8,
                                    kind="ExternalInput", name="xn8",
                                    uniquify=False)
                else:
                    xn8 = dram.tile([N8 * P, 2, O_SHARD], f8,
                                    kind="ExternalInput", name="xn8",
                                    uniquify=False)
                basem = dram.tile([P, T_SHARD], f32,
                                  kind="ExternalInput", name="basem",
                                  uniquify=False)
            if KB:
                xm16 = dram.tile([KB, T_SHARD], bf16,
                                 kind="ExternalInput", name="xm16",
                                 uniquify=False)
                xn16 = dram.tile([KB, O_SHARD], bf16,
                                 kind="ExternalInput", name="xn16",
                                 uniquify=False)
                xm16v = xm16[:].rearrange("(kb p) m -> p kb m", p=P)
            yt = dram.tile([O_SHARD, T_SHARD], f32,
                           kind="ExternalOutput", name="yt", uniquify=False)

            with tc.tile_pool(name="xpool", bufs=N8 + KB_T + 1) as xpool, \
                 tc.tile_pool(name="wpool",
                              bufs=(N8 + KB_T + 4) if STRIP else 8) as wpool, \
                 tc.tile_pool(name="pspool", bufs=2, space="PSUM") as pspool, \
                 tc.tile_pool(name="evpool", bufs=8) as evpool:
                xt8 = [None] * N8
                xt16 = [None] * KB_T
                base_sb = None
                NG = O_SHARD // P   # 16 groups of 128 output features
                first = True
                for _ in range(repeats):
                    for ng in range(NG):
                        banks = [pspool.tile([P, 512], f32,
                                             name=f"bank_{mc}",
                                             tag=f"bank_{mc}")
                                 for mc in range(MC)]
                        if first and N8:
                            base_sb = xpool.tile([P, T_SHARD], f32,
                                                 name="base_sb", tag="xt")
                            nc.sync.dma_start(base_sb[:], basem[:])
                        for kt in range(N8):
                            if SWI:
                                wt8 = wpool.tile([P, 2 * P], f8,
                                                 name="wt8", tag="wt")
                                src = xn8[kt * P:(kt + 1) * P, ng, :]
                            else:
                                wt8 = wpool.tile([P, 2, P], f8,
                                                 name="wt8", tag="wt")
                                src = xn8[kt * P:(kt + 1) * P, :,
                                          ng * P:(ng + 1) * P]
                            nc.sync.dma_start(wt8[:], src)
                            if first:
                                xt = xpool.tile(
                                    [P, T_SHARD, 2] if XI else [P, 2, T_SHARD],
                                    f8, name="xt8", tag="xt")
                                nc.sync.dma_start(
                                    xt[:], xm8[kt * P:(kt + 1) * P, :, :])
                                if XI:
                                    xt8[kt] = xt[:].rearrange(
                                        "p m two -> p two m")
                                else:
                                    xt8[kt] = xt
                            for mc in range(MC):
                                nc.tensor.matmul(
                                    banks[mc][:],
                                    wt8[:],
                                    xt8[kt][:, :, mc * 512:(mc + 1) * 512],
                                    start=(kt == 0),
                                    stop=(KB == 0 and kt == N8 - 1),
                                    perf_mode=dr_mode,
                                )
                        for kb in range(KB_T):
                            wt = wpool.tile([P, P], bf16,
                                            name="wt", tag="wt")
                            nc.sync.dma_start(
                                wt[:], xn16[kb * P:(kb + 1) * P,
                                            ng * P:(ng + 1) * P])
                            if first:
                                xt = xpool.tile([P, T_SHARD], bf16,
                                                name="xt16", tag="xt")
                                nc.sync.dma_start(xt[:], xm16v[:, kb])
                                xt16[kb] = xt
                            for mc in range(MC):
                                nc.tensor.matmul(
                                    banks[mc][:],
                                    wt[:],
                                    xt16[kb][:, mc * 512:(mc + 1) * 512],
                                    start=(N8 == 0 and kb == 0),
                                    stop=(kb == KB_T - 1),
                                )
                        first = False
                        for mc in range(MC):
                            ev = evpool.tile([P, 512], f32,
                                             name="ev", tag="ev")
                            if N8:
                                nc.vector.scalar_tensor_tensor(
                                    out=ev[:],
                                    in0=banks[mc][:],
                                    scalar=0.0,
                                    in1=base_sb[:, mc * 512:(mc + 1) * 512],
                                    op0=mybir.AluOpType.bypass,
                                    op1=mybir.AluOpType.add,
                                )
                            else:
                                nc.vector.tensor_copy(
                                    out=ev[:], in_=banks[mc][:])
                            nc.sync.dma_start(
                                yt[ng * P:(ng + 1) * P,
                                   mc * 512:(mc + 1) * 512],
                                ev[:])
    _dedupe_ldweights(nc.m)
    if STRIP:
        _strip_mm_sem_incs(nc.m)
    nc.compile()
    _NC_CACHE[key] = nc
    return nc


def prepare_in_maps(x: np.ndarray, weight: np.ndarray):
    """Host-side shard prep; see module docstring for the layout."""
    bf16 = ml_dtypes.bfloat16
    f8 = ml_dtypes.float8_e4m3
    x2 = np.ascontiguousarray(np.asarray(x).reshape(TOKENS, D_IN))
    w = np.asarray(weight)

    xm8_full = xn8_os = base = None
    xm16_full = xn16_os = None
    if N8:
        xs8 = x2[:, :KF8].astype(f8)  # [TOKENS, KF8]
        v8 = (w[:, :KF8].astype(np.float32) - 128.0).astype(f8)  # [D_OUT,KF8]
        base = (128.0 * x2[:, :KF8].astype(np.float64).sum(axis=1)
                ).astype(np.float32)  # [TOKENS]
        # row r = kt*128+p maps to contraction dim kt*256 + i*128 + p
        if XI:
            xm8_full = np.ascontiguousarray(
                xs8.T.reshape(N8, 2, P, TOKENS).transpose(0, 2, 3, 1)
                .reshape(N8 * P, TOKENS, 2))
        else:
            xm8_full = np.ascontiguousarray(
                xs8.T.reshape(N8, 2, P, TOKENS).transpose(0, 2, 1, 3)
                .reshape(N8 * P, 2, TOKENS))
        w8 = (v8.T.reshape(N8, 2, P, D_OUT).transpose(0, 2, 1, 3)
              .reshape(N8 * P, 2, D_OUT))  # [row, plane, out]
        if SWI:
            # per 128-out block: pairs (plane0, plane1) interleaved with
            # the block's output columns reversed:
            #   elem[2j+i] = plane_i[127 - j]
            w8b = w8.reshape(N8 * P, 2, D_OUT // P, P)  # [row, i, blk, o]
            w8b = w8b[:, :, :, ::-1]                    # reverse cols
            w8b = w8b.transpose(0, 2, 3, 1)             # [row, blk, j, i]
            w8swi = w8b.reshape(N8 * P, D_OUT // P, 2 * P)
            nblk = O_SHARD // P
            xn8_os = [
                np.ascontiguousarray(w8swi[:, o * nblk:(o + 1) * nblk, :])
                for o in range(OS)
            ]
        else:
            xn8_os = [
                np.ascontiguousarray(w8[:, :, o * O_SHARD:(o + 1) * O_SHARD])
                for o in range(OS)
            ]
    if KB:
        xm16_full = np.ascontiguousarray(
            x2[:, KF8:].astype(bf16).T)  # [KB, TOKENS]
        wt16 = w[:, KF8:].astype(np.float32).astype(bf16).T  # [KB, D_OUT]
        xn16_os = [
            np.ascontiguousarray(wt16[:, o * O_SHARD:(o + 1) * O_SHARD])
            for o in range(OS)
        ]

    in_maps = []
    for c in range(N_CORES):
        ts, osd = c % TS, c // TS
        sl = slice(ts * T_SHARD, (ts + 1) * T_SHARD)
        m = {}
        if N8:
            m["xm8"] = np.ascontiguousarray(
                xm8_full[:, sl, :] if XI else xm8_full[:, :, sl])
            m["xn8"] = xn8_os[osd]
            m["basem"] = np.ascontiguousarray(
                np.broadcast_to(base[sl], (P, T_SHARD)))
        if KB:
            m["xm16"] = np.ascontiguousarray(xm16_full[:, sl])
            m["xn16"] = xn16_os[osd]
        in_maps.append(m)
    return in_maps


def gather_output(results):
    y = np.empty((TOKENS, D_OUT), dtype=np.float32)
    for c in range(N_CORES):
        ts, osd = c % TS, c // TS
        y[ts * T_SHARD:(ts + 1) * T_SHARD,
          osd * O_SHARD:(osd + 1) * O_SHARD] = results[c]["yt"].T
    return y.reshape(4, 2048, D_OUT)


def kernel(x: np.ndarray, weight: np.ndarray) -> np.ndarray:
    from concourse.bass_utils import run_bass_kernel_spmd

    nc = build_nc()
    in_maps = prepare_in_maps(x, weight)
    res = run_bass_kernel_spmd(nc, in_maps, core_ids=list(range(N_CORES)))
    return gather_output(res.results)
